# revision 2
# baseline (speedup 1.0000x reference)
"""Multi-head attention kernel for Trainium2 (Bass/Tile), 8 NeuronCores.

Problem: nn_MultiHeadAttention  (B=4, S=2048, D=1024, H=16, DK=64)
    out = softmax((q Wq^T + bq)(k Wk^T + bk)^T / sqrt(DK)) (v Wv^T + bv) Wo^T + bo

Sharding: core c = 2*b + g handles batch b and head-group g (8 heads = 512
features).  Each core computes its batch's attention for its heads plus a
partial output projection; the host sums the two partials per batch.

Math simplifications done on the host (exact):
  - k-bias bk drops out (softmax is shift invariant along the key axis).
  - v-bias bv folds into an effective output bias bo_eff = bo + Wo @ bv.
  - the 1/sqrt(DK) logit scale is folded into Wq/bq.

v2 on-chip structure (vs v1):
  - Heads are processed in PAIRS (even head at SBUF partitions 0-63, odd at
    64-127).  The QK^T matmuls have K=64 contraction, so the pair's matmuls
    row-tile the PE array (tile_position (0,0) and (64,0)) and run
    CONCURRENTLY on the two 64-row halves -> QK phase time halved.
  - S^T PSUM slots: ring of 3x [128, 1024] (6 banks).  Per (si, half) step
    the pair's 4 QK matmuls (A f0, B f0, A f1, B f1) write two slots; exp
    order (B then A) + slot-assignment makes ScalarE run gapless while the
    ring stays only 3 deep.
  - PV (O^T = V^T E, M=65 with a ones-column giving the softmax denominator
    in row 64) trails the exp stream in 4-si blocks: accumulate 4 key-chunks
    in 2 PSUM banks (window f0/f1 then f2/f3), then DVE-add into an SBUF
    fp32 accumulator `ou`.  E tiles free quickly -> small E pool.
  - Projections (Q/K/V/out) use a 2-deep [128,512] PSUM ring; the
    dependency-driven Tile scheduler runs them in PE idle gaps of the
    ScalarE-bound attention phases.
"""

import numpy as np
import ml_dtypes
from contextlib import ExitStack

import concourse.bass as bass
import concourse.tile as tile
from concourse import bacc, mybir
from concourse.bass import ts, ds
from concourse.bass_utils import run_bass_kernel_spmd

B, S, D, H, DK = 4, 2048, 1024, 16, 64
N_CORES = 8
F32 = mybir.dt.float32
BF16 = mybir.dt.bfloat16
AF = mybir.ActivationFunctionType
ALU = mybir.AluOpType
BF16NP = ml_dtypes.bfloat16


def build_nc(s: int = S):
    """Build + compile the per-core Bass module (SPMD: same NEFF, per-core data)."""
    assert s == 2048, "v2 kernel is specialized to S=2048"
    nsi = s // 128   # 128-row key chunks (16)
    nf = s // 512    # 512-col query chunks (4)
    BLK = 4          # si chunks per PV accumulation block

    nc = bacc.Bacc("TRN2", target_bir_lowering=False, debug=False)

    qT = nc.dram_tensor("qT", [D, s], BF16, kind="ExternalInput").ap()
    kT = nc.dram_tensor("kT", [D, s], BF16, kind="ExternalInput").ap()
    vT = nc.dram_tensor("vT", [D, s], BF16, kind="ExternalInput").ap()
    wq = nc.dram_tensor("wq", [D, 512], BF16, kind="ExternalInput").ap()
    wk = nc.dram_tensor("wk", [D, 512], BF16, kind="ExternalInput").ap()
    wv = nc.dram_tensor("wv", [D, 512], BF16, kind="ExternalInput").ap()
    wo = nc.dram_tensor("wo", [512, D], BF16, kind="ExternalInput").ap()
    bq = nc.dram_tensor("bq", [128, 4], F32, kind="ExternalInput").ap()
    outT = nc.dram_tensor("outT", [D, s], F32, kind="ExternalOutput").ap()

    with tile.TileContext(nc) as tc, ExitStack() as ctx:
        pers = ctx.enter_context(tc.tile_pool(name="pers", bufs=1))
        # S^T slots: 3 x [128,1024] = 6 PSUM banks
        spool = ctx.enter_context(tc.tile_pool(name="spool", bufs=3, space="PSUM"))
        # accumulator / projection ring: 2 x [128,512] = 2 PSUM banks
        apool = ctx.enter_context(tc.tile_pool(name="apool", bufs=2, space="PSUM"))
        epool = ctx.enter_context(tc.tile_pool(name="e", bufs=34))
        xpool = ctx.enter_context(tc.tile_pool(name="x", bufs=16))
        wpool = ctx.enter_context(tc.tile_pool(name="w", bufs=1))
        oupool = ctx.enter_context(tc.tile_pool(name="ou", bufs=2))
        bpool = ctx.enter_context(tc.tile_pool(name="b", bufs=2))
        dpool = ctx.enter_context(tc.tile_pool(name="dscr", bufs=4, space="DRAM"))
        opool = ctx.enter_context(tc.tile_pool(name="ostage", bufs=3))

        QT = pers.tile([128, 4, s], BF16)       # Q'^T  [feature, seq], pair-major
        KT = pers.tile([128, 4, s], BF16)       # K^T   [feature, seq]
        V = pers.tile([128, nsi, 8, 66], BF16)  # V nat [seq, head, dv|ones|pad]
        O = pers.tile([128, 4, s], BF16)        # O^T normalized
        WO = pers.tile([128, 4, D], BF16)
        BQ = pers.tile([128, 4], F32)

        nc.sync.dma_start(WO[:], wo.rearrange("(o p) e -> p o e", p=128))
        nc.sync.dma_start(BQ[:], bq)
        nc.vector.memset(V[:, :, :, 64:65], 1.0)

        # ---- projections: Q' (scaled+bias), K ---------------------------
        for xdram, wdram, dst, bias in ((qT, wq, QT, BQ), (kT, wk, KT, None)):
            wt = wpool.tile([128, 8, 512], BF16, tag="w")
            nc.sync.dma_start(wt[:], wdram.rearrange("(o p) m -> p o m", p=128))
            for f in range(nf):
                xts = []
                for ki in range(8):
                    xt = xpool.tile([128, 512], BF16, tag="x")
                    nc.sync.dma_start(
                        xt[:], xdram[ds(ki * 128, 128), ds(f * 512, 512)]
                    )
                    xts.append(xt)
                for pc in range(4):
                    ps = apool.tile([128, 512], F32, tag="a")
                    for ki in range(8):
                        nc.tensor.matmul(
                            ps[:],
                            lhsT=wt[:, ki, ts(pc, 128)],
                            rhs=xts[ki][:],
                            start=(ki == 0),
                            stop=(ki == 7),
                        )
                    if bias is not None:
                        nc.vector.tensor_scalar_add(
                            dst[:, pc, ts(f, 512)], ps[:], bias[:, pc : pc + 1]
                        )
                    else:
                        nc.vector.tensor_copy(dst[:, pc, ts(f, 512)], ps[:])

        # ---- V projection ----------------------------------------------
        wt = wpool.tile([128, 8, 512], BF16, tag="w")
        nc.sync.dma_start(wt[:], wv.rearrange("(o p) m -> p o m", p=128))
        for f in range(nf):
            xts = []
            for ki in range(8):
                xt = xpool.tile([128, 512], BF16, tag="x")
                nc.sync.dma_start(xt[:], vT[ds(ki * 128, 128), ds(f * 512, 512)])
                xts.append(xt)
            for sj in range(4):
                si = f * 4 + sj
                vps = apool.tile([128, 512], F32, tag="a")
                for ki in range(8):
                    nc.tensor.matmul(
                        vps[:],
                        lhsT=xts[ki][:, ts(sj, 128)],
                        rhs=wt[:, ki, :],
                        start=(ki == 0),
                        stop=(ki == 7),
                    )
                nc.vector.tensor_copy(
                    V[:, si, :, 0:64],
                    vps[:].rearrange("p (h d) -> p h d", h=8),
                )

        # ---- attention phases: one per head pair -----------------------
        def pv_block(hp, hh, h, blk, es, ou):
            """PV for one head over si block [blk*BLK, blk*BLK+BLK):
            two windows of 2 f-chunks, accumulated in PSUM then DVE-merged
            into the SBUF fp32 accumulator `ou` [65 rows: O^T | denom]."""
            for w in range(2):
                accs = [
                    apool.tile([128, 512], F32, tag="a", name=f"pv_{h}_{blk}_{w}_{i}")
                    for i in range(2)
                ]
                for sj in range(BLK):
                    si = blk * BLK + sj
                    for fo in range(2):
                        f = 2 * w + fo
                        nc.tensor.matmul(
                            accs[fo][0:65, :],
                            lhsT=V[:, si, h, 0:65],
                            rhs=es[si][w][:, ts(fo, 512)],
                            start=(sj == 0),
                            stop=(sj == BLK - 1),
                        )
                for fo in range(2):
                    f = 2 * w + fo
                    if blk == 0:
                        nc.vector.tensor_copy(
                            ou[0:65, ts(f, 512)], accs[fo][0:65, :]
                        )
                    else:
                        nc.vector.tensor_tensor(
                            ou[0:65, ts(f, 512)],
                            ou[0:65, ts(f, 512)],
                            accs[fo][0:65, :],
                            ALU.add,
                        )

        def pv_finish(hp, hh, h, ou):
            """Normalize: reciprocal of the denominator (row 64) on a
            16-lane DMA-reshaped view, partition-broadcast via step-0 DMA
            read, multiply into O."""
            dscr = dpool.tile([1, s], F32, tag="dscr", name=f"dscr_{h}")
            nc.sync.dma_start(dscr[:], ou[ds(64, 1), :])
            d16 = bpool.tile([16, s // 16], F32, tag="d16", name=f"d16_{h}")
            nc.sync.dma_start(
                d16[:], dscr[:].rearrange("one (p c) -> (one p) c", p=16)
            )
            r16 = bpool.tile([16, s // 16], F32, tag="r16", name=f"r16_{h}")
            nc.vector.reciprocal(r16[:], d16[:])
            dsc2 = dpool.tile([1, s], F32, tag="dsc2", name=f"dsc2_{h}")
            nc.sync.dma_start(
                dsc2[:].rearrange("one (p c) -> (one p) c", p=16), r16[:]
            )
            bsb = bpool.tile([64, s], F32, tag="bsb", name=f"bsb_{h}")
            nc.sync.dma_start(bsb[:], dsc2[:].to_broadcast((64, s)))
            for f in range(nf):
                nc.vector.tensor_tensor(
                    O[ds(hh, 64), hp, ts(f, 512)],
                    ou[0:64, ts(f, 512)],
                    bsb[0:64, ts(f, 512)],
                    ALU.mult,
                )

        for hp in range(4):  # head pair: heads hA=2*hp (parts 0-63), hB (64-127)
            hA, hB = 2 * hp, 2 * hp + 1
            esA = [[None, None] for _ in range(nsi)]  # E tiles [si][half]
            esB = [[None, None] for _ in range(nsi)]
            ouA = oupool.tile([65, s], F32, tag="ou", name=f"ou_{hA}")
            ouB = oupool.tile([65, s], F32, tag="ou", name=f"ou_{hB}")
            for si in range(nsi):
                for half in range(2):
                    # slot-ring discipline for gapless ScalarE (see header):
                    # allocate B slot then A slot; emit A's matmuls first
                    # (interleaved A,B,A,B); exp B first, then A.
                    sB = spool.tile(
                        [128, 1024], F32, tag="s", name=f"sB_{hp}_{si}_{half}"
                    )
                    sA = spool.tile(
                        [128, 1024], F32, tag="s", name=f"sA_{hp}_{si}_{half}"
                    )
                    for fo in range(2):
                        f = 2 * half + fo
                        nc.tensor.matmul(
                            sA[:, ts(fo, 512)],
                            lhsT=KT[ds(0, 64), hp, ts(si, 128)],
                            rhs=QT[ds(0, 64), hp, ts(f, 512)],
                            start=True,
                            stop=True,
                        )
                        nc.tensor.matmul(
                            sB[:, ts(fo, 512)],
                            lhsT=KT[ds(64, 64), hp, ts(si, 128)],
                            rhs=QT[ds(64, 64), hp, ts(f, 512)],
                            start=True,
                            stop=True,
                        )
                    eB = epool.tile(
                        [128, 1024], BF16, tag="e", name=f"e_{hB}_{si}_{half}"
                    )
                    nc.scalar.activation(eB[:], sB[:], AF.Exp)
                    eA = epool.tile(
                        [128, 1024], BF16, tag="e", name=f"e_{hA}_{si}_{half}"
                    )
                    nc.scalar.activation(eA[:], sA[:], AF.Exp)
                    esA[si][half] = eA
                    esB[si][half] = eB
                if si % BLK == BLK - 1:
                    blk = si // BLK
                    pv_block(hp, 0, hA, blk, esA, ouA)
                    pv_block(hp, 64, hB, blk, esB, ouB)
            pv_finish(hp, 0, hA, ouA)
            pv_finish(hp, 64, hB, ouB)

        # ---- output projection (partial over this core's heads) --------
        outr = outT.rearrange("(o p) n -> p o n", p=128)
        for pe in range(8):
            for f in range(nf):
                ps = apool.tile([128, 512], F32, tag="a")
                for ki in range(4):
                    nc.tensor.matmul(
                        ps[:],
                        lhsT=WO[:, ki, ts(pe, 128)],
                        rhs=O[:, ki, ts(f, 512)],
                        start=(ki == 0),
                        stop=(ki == 3),
                    )
                ot = opool.tile([128, 512], F32, tag="ot")
                nc.vector.tensor_copy(ot[:], ps[:])
                nc.sync.dma_start(outr[:, pe, ts(f, 512)], ot[:])

    nc.compile()
    return nc


_NC_CACHE: dict = {}


def get_nc(s: int = S):
    if s not in _NC_CACHE:
        _NC_CACHE[s] = build_nc(s)
    return _NC_CACHE[s]


def _prep_in_maps(q, k, v, Wq, bq, Wk, Wv, Wo):
    """Host-side shard prep: per-core input dicts (cheap numpy reshapes)."""
    f32 = np.float32
    scale = 1.0 / np.sqrt(DK)
    xT = {}
    for b in range(B):
        xT[b] = (
            np.ascontiguousarray(q[b].T).astype(BF16NP),
            np.ascontiguousarray(k[b].T).astype(BF16NP),
            np.ascontiguousarray(v[b].T).astype(BF16NP),
        )
    per_g = {}
    for g in range(2):
        F = slice(512 * g, 512 * g + 512)
        per_g[g] = dict(
            wq=np.ascontiguousarray(Wq[F].T * scale).astype(BF16NP),
            wk=np.ascontiguousarray(Wk[F].T).astype(BF16NP),
            wv=np.ascontiguousarray(Wv[F].T).astype(BF16NP),
            wo=np.ascontiguousarray(Wo[:, F].T).astype(BF16NP),
            bq=np.ascontiguousarray(
                (bq[F] * scale).reshape(4, 128).T, dtype=f32
            ),
        )
    in_maps = []
    for c in range(N_CORES):
        b, g = c // 2, c % 2
        qb, kb, vb = xT[b]
        in_maps.append(dict(qT=qb, kT=kb, vT=vb, **per_g[g]))
    return in_maps


def kernel(q, k, v, Wq, bq, Wk, bk, Wv, bv, Wo, bo):
    q, k, v = (np.asarray(x, np.float32) for x in (q, k, v))
    Wq, bq, Wk, bk = (np.asarray(x, np.float32) for x in (Wq, bq, Wk, bk))
    Wv, bv, Wo, bo = (np.asarray(x, np.float32) for x in (Wv, bv, Wo, bo))

    nc = get_nc(S)
    in_maps = _prep_in_maps(q, k, v, Wq, bq, Wk, Wv, Wo)
    res = run_bass_kernel_spmd(nc, in_maps, core_ids=list(range(N_CORES)))

    # bk drops out of softmax; bv folds into an effective output bias.
    bo_eff = (
        bo.astype(np.float64) + Wo.astype(np.float64) @ bv.astype(np.float64)
    ).astype(np.float32)
    out = np.empty((B, S, D), np.float32)
    for b in range(B):
        acc = res.results[2 * b]["outT"] + res.results[2 * b + 1]["outT"]
        out[b] = acc.T + bo_eff
    return out


# revision 4
# speedup vs baseline: 1.2084x; 1.2084x over previous
"""Multi-head attention kernel for Trainium2 (Bass/Tile), 8 NeuronCores.

Problem: nn_MultiHeadAttention  (B=4, S=2048, D=1024, H=16, DK=64)
    out = softmax((q Wq^T + bq)(k Wk^T + bk)^T / sqrt(DK)) (v Wv^T + bv) Wo^T + bo

Sharding: core c = 2*b + g handles batch b and head-group g (8 heads = 512
features).  Each core computes its batch's attention for its heads plus a
partial output projection; the host sums the two partials per batch.

Math simplifications done on the host (exact):
  - k-bias bk drops out (softmax is shift invariant along the key axis).
  - v-bias bv folds into an effective output bias bo_eff = bo + Wo @ bv.
  - the 1/sqrt(DK) logit scale is folded into Wq/bq.

v3 on-chip structure:
  - Heads are processed in PAIRS (even head at SBUF partitions 0-63, odd at
    64-127).  The QK^T matmuls have K=64 contraction, so the pair's two
    matmuls row-tile the PE array (tile_position (0,0) / (64,0)) and run
    concurrently on the two 64-row halves.  To make both matmuls of a pair
    become READY at the same instant (the dependency-driven Tile scheduler
    otherwise de-pairs them), both write the SAME PSUM slot: head A fills
    columns 0:512, head B columns 512:1024 of one [128,1024] slot (two
    different banks), gated by one ring release; a single exp covers both.
  - S^T PSUM slots: ring of 3x [128,1024] (6 banks).  exp per (si, f-chunk)
    -> E tile [128, A512|B512] bf16 in SBUF.
  - PV (O^T = V^T E, M=65, ones-column = softmax denominator in row 64)
    trails the exp stream in 4-si blocks: window f{0,1} then f{2,3}
    accumulates in 2 PSUM banks, then DVE-merges into an SBUF fp32
    accumulator `ou` per head.  E tiles free quickly -> small E pool.
  - Q/K projections interleave per f-chunk so pair 0's first QK matmuls are
    ready after ~5 projection groups (exp stream starts at ~15us, not 85us).
    V-projection groups are spread through pair 0's phase.
"""

import numpy as np
import ml_dtypes
from contextlib import ExitStack

import concourse.bass as bass
import concourse.tile as tile
from concourse import bacc, mybir
from concourse.bass import ts, ds
from concourse.bass_utils import run_bass_kernel_spmd

B, S, D, H, DK = 4, 2048, 1024, 16, 64
N_CORES = 8
F32 = mybir.dt.float32
BF16 = mybir.dt.bfloat16
AF = mybir.ActivationFunctionType
ALU = mybir.AluOpType
BF16NP = ml_dtypes.bfloat16


def build_nc(s: int = S):
    """Build + compile the per-core Bass module (SPMD: same NEFF, per-core data)."""
    assert s == 2048, "kernel is specialized to S=2048"
    nsi = s // 128   # 128-row key chunks (16)
    nf = s // 512    # 512-col query chunks (4)
    BLK = 4          # si chunks per PV accumulation block

    nc = bacc.Bacc("TRN2", target_bir_lowering=False, debug=False)

    qT = nc.dram_tensor("qT", [D, s], BF16, kind="ExternalInput").ap()
    kT = nc.dram_tensor("kT", [D, s], BF16, kind="ExternalInput").ap()
    vT = nc.dram_tensor("vT", [D, s], BF16, kind="ExternalInput").ap()
    wq = nc.dram_tensor("wq", [D, 512], BF16, kind="ExternalInput").ap()
    wk = nc.dram_tensor("wk", [D, 512], BF16, kind="ExternalInput").ap()
    wv = nc.dram_tensor("wv", [D, 512], BF16, kind="ExternalInput").ap()
    wo = nc.dram_tensor("wo", [512, D], BF16, kind="ExternalInput").ap()
    bq = nc.dram_tensor("bq", [128, 4], F32, kind="ExternalInput").ap()
    outT = nc.dram_tensor("outT", [D, s], F32, kind="ExternalOutput").ap()

    with tile.TileContext(nc) as tc, ExitStack() as ctx:
        pers = ctx.enter_context(tc.tile_pool(name="pers", bufs=1))
        # S^T slots: 3 x [128,1024] = 6 PSUM banks
        spool = ctx.enter_context(tc.tile_pool(name="spool", bufs=3, space="PSUM"))
        # accumulator / projection ring: 2 x [128,512] = 2 PSUM banks
        apool = ctx.enter_context(tc.tile_pool(name="apool", bufs=2, space="PSUM"))
        epool = ctx.enter_context(tc.tile_pool(name="e", bufs=32))
        xpool = ctx.enter_context(tc.tile_pool(name="x", bufs=16))
        wpool = ctx.enter_context(tc.tile_pool(name="w", bufs=2))
        oupool = ctx.enter_context(tc.tile_pool(name="ou", bufs=2))
        bpool = ctx.enter_context(tc.tile_pool(name="b", bufs=2))
        dpool = ctx.enter_context(tc.tile_pool(name="dscr", bufs=4, space="DRAM"))
        opool = ctx.enter_context(tc.tile_pool(name="ostage", bufs=2))

        QT = pers.tile([128, 4, s], BF16)       # Q'^T  [feature, seq], pair-major
        KT = pers.tile([128, 4, s], BF16)       # K^T   [feature, seq]
        V = pers.tile([128, nsi, 8, 66], BF16)  # V nat [seq, head, dv|ones|pad]
        O = pers.tile([128, 4, s], BF16)        # O^T normalized
        WO = pers.tile([128, 4, D], BF16)
        BQ = pers.tile([128, 4], F32)

        nc.sync.dma_start(WO[:], wo.rearrange("(o p) e -> p o e", p=128))
        nc.sync.dma_start(BQ[:], bq)
        nc.vector.memset(V[:, :, :, 64:65], 1.0)

        # ---- Q'/K projections, interleaved per f-chunk ------------------
        wtq = wpool.tile([128, 8, 512], BF16, tag="w", name="wtq")
        nc.sync.dma_start(wtq[:], wq.rearrange("(o p) m -> p o m", p=128))
        wtk = wpool.tile([128, 8, 512], BF16, tag="w", name="wtk")
        nc.sync.dma_start(wtk[:], wk.rearrange("(o p) m -> p o m", p=128))
        for f in range(nf):
            for xdram, wt, dst, bias in (
                (qT, wtq, QT, BQ),
                (kT, wtk, KT, None),
            ):
                xts = []
                for ki in range(8):
                    xt = xpool.tile([128, 512], BF16, tag="x")
                    nc.sync.dma_start(
                        xt[:], xdram[ds(ki * 128, 128), ds(f * 512, 512)]
                    )
                    xts.append(xt)
                for pc in range(4):
                    ps = apool.tile([128, 512], F32, tag="a")
                    for ki in range(8):
                        nc.tensor.matmul(
                            ps[:],
                            lhsT=wt[:, ki, ts(pc, 128)],
                            rhs=xts[ki][:],
                            start=(ki == 0),
                            stop=(ki == 7),
                        )
                    if bias is not None:
                        nc.vector.tensor_scalar_add(
                            dst[:, pc, ts(f, 512)], ps[:], bias[:, pc : pc + 1]
                        )
                    else:
                        nc.vector.tensor_copy(dst[:, pc, ts(f, 512)], ps[:])

        wtv = wpool.tile([128, 8, 512], BF16, tag="w", name="wtv")
        nc.sync.dma_start(wtv[:], wv.rearrange("(o p) m -> p o m", p=128))

        def vproj_group(si):
            """One V-projection group: V rows for key chunk si (all 8 heads)."""
            f, sj = si // 4, si % 4
            if sj == 0:
                xts = []
                for ki in range(8):
                    xt = xpool.tile([128, 512], BF16, tag="x")
                    nc.sync.dma_start(
                        xt[:], vT[ds(ki * 128, 128), ds(f * 512, 512)]
                    )
                    xts.append(xt)
                vproj_group.xts = xts
            vps = apool.tile([128, 512], F32, tag="a")
            for ki in range(8):
                nc.tensor.matmul(
                    vps[:],
                    lhsT=vproj_group.xts[ki][:, ts(sj, 128)],
                    rhs=wtv[:, ki, :],
                    start=(ki == 0),
                    stop=(ki == 7),
                )
            nc.vector.tensor_copy(
                V[:, si, :, 0:64],
                vps[:].rearrange("p (h d) -> p h d", h=8),
            )

        # ---- attention phases: one per head pair -----------------------
        def pv_block(h, hsel, blk, es, ou):
            """PV for one head over si block [blk*BLK, (blk+1)*BLK): two
            windows of 2 f-chunks accumulate in PSUM, then DVE-merge into
            the SBUF fp32 accumulator `ou` [65 rows: O^T | denom]."""
            for w in range(2):
                accs = [
                    apool.tile([128, 512], F32, tag="a", name=f"pv_{h}_{blk}_{w}_{i}")
                    for i in range(2)
                ]
                for sj in range(BLK):
                    si = blk * BLK + sj
                    for fo in range(2):
                        f = 2 * w + fo
                        nc.tensor.matmul(
                            accs[fo][0:65, :],
                            lhsT=V[:, si, h, 0:65],
                            rhs=es[si][f][:, ts(hsel, 512)],
                            start=(sj == 0),
                            stop=(sj == BLK - 1),
                        )
                for fo in range(2):
                    f = 2 * w + fo
                    if blk == 0:
                        nc.vector.tensor_copy(
                            ou[0:65, ts(f, 512)], accs[fo][0:65, :]
                        )
                    else:
                        nc.vector.tensor_tensor(
                            ou[0:65, ts(f, 512)],
                            ou[0:65, ts(f, 512)],
                            accs[fo][0:65, :],
                            ALU.add,
                        )

        def pv_finish(hp, hh, h, ou):
            """Normalize: reciprocal of the denominator (row 64) on a
            16-lane DMA-reshaped view, partition-broadcast via step-0 DMA
            read, multiply into O."""
            dscr = dpool.tile([1, s], F32, tag="dscr", name=f"dscr_{h}")
            nc.sync.dma_start(dscr[:], ou[ds(64, 1), :])
            d16 = bpool.tile([16, s // 16], F32, tag="d16", name=f"d16_{h}")
            nc.sync.dma_start(
                d16[:], dscr[:].rearrange("one (p c) -> (one p) c", p=16)
            )
            r16 = bpool.tile([16, s // 16], F32, tag="r16", name=f"r16_{h}")
            nc.vector.reciprocal(r16[:], d16[:])
            dsc2 = dpool.tile([1, s], F32, tag="dsc2", name=f"dsc2_{h}")
            nc.sync.dma_start(
                dsc2[:].rearrange("one (p c) -> (one p) c", p=16), r16[:]
            )
            bsb = bpool.tile([64, s], F32, tag="bsb", name=f"bsb_{h}")
            nc.sync.dma_start(bsb[:], dsc2[:].to_broadcast((64, s)))
            for f in range(nf):
                nc.vector.tensor_tensor(
                    O[ds(hh, 64), hp, ts(f, 512)],
                    ou[0:64, ts(f, 512)],
                    bsb[0:64, ts(f, 512)],
                    ALU.mult,
                )

        for hp in range(4):  # head pair: hA=2*hp (parts 0-63), hB (64-127)
            hA, hB = 2 * hp, 2 * hp + 1
            es = [[None] * nf for _ in range(nsi)]  # E tiles [si][f] = [A|B]
            ouA = oupool.tile([65, s], F32, tag="ou", name=f"ou_{hA}")
            ouB = oupool.tile([65, s], F32, tag="ou", name=f"ou_{hB}")
            for si in range(nsi):
                if hp == 0:
                    vproj_group(si)
                for f in range(nf):
                    sl = spool.tile(
                        [128, 1024], F32, tag="s", name=f"s_{hp}_{si}_{f}"
                    )
                    nc.tensor.matmul(
                        sl[:, 0:512],
                        lhsT=KT[ds(0, 64), hp, ts(si, 128)],
                        rhs=QT[ds(0, 64), hp, ts(f, 512)],
                        start=True,
                        stop=True,
                    )
                    nc.tensor.matmul(
                        sl[:, 512:1024],
                        lhsT=KT[ds(64, 64), hp, ts(si, 128)],
                        rhs=QT[ds(64, 64), hp, ts(f, 512)],
                        start=True,
                        stop=True,
                    )
                    e = epool.tile(
                        [128, 1024], BF16, tag="e", name=f"e_{hp}_{si}_{f}"
                    )
                    nc.scalar.activation(e[:], sl[:], AF.Exp)
                    es[si][f] = e
                if si % BLK == BLK - 1:
                    blk = si // BLK
                    pv_block(hA, 0, blk, es, ouA)
                    pv_block(hB, 1, blk, es, ouB)
            pv_finish(hp, 0, hA, ouA)
            pv_finish(hp, 64, hB, ouB)

        # ---- output projection (partial over this core's heads) --------
        outr = outT.rearrange("(o p) n -> p o n", p=128)
        for pe in range(8):
            for f in range(nf):
                ps = apool.tile([128, 512], F32, tag="a")
                for ki in range(4):
                    nc.tensor.matmul(
                        ps[:],
                        lhsT=WO[:, ki, ts(pe, 128)],
                        rhs=O[:, ki, ts(f, 512)],
                        start=(ki == 0),
                        stop=(ki == 3),
                    )
                ot = opool.tile([128, 512], F32, tag="ot")
                nc.vector.tensor_copy(ot[:], ps[:])
                nc.sync.dma_start(outr[:, pe, ts(f, 512)], ot[:])

    nc.compile()
    return nc


_NC_CACHE: dict = {}


def get_nc(s: int = S):
    if s not in _NC_CACHE:
        _NC_CACHE[s] = build_nc(s)
    return _NC_CACHE[s]


def _prep_in_maps(q, k, v, Wq, bq, Wk, Wv, Wo):
    """Host-side shard prep: per-core input dicts (cheap numpy reshapes)."""
    f32 = np.float32
    scale = 1.0 / np.sqrt(DK)
    xT = {}
    for b in range(B):
        xT[b] = (
            np.ascontiguousarray(q[b].T).astype(BF16NP),
            np.ascontiguousarray(k[b].T).astype(BF16NP),
            np.ascontiguousarray(v[b].T).astype(BF16NP),
        )
    per_g = {}
    for g in range(2):
        F = slice(512 * g, 512 * g + 512)
        per_g[g] = dict(
            wq=np.ascontiguousarray(Wq[F].T * scale).astype(BF16NP),
            wk=np.ascontiguousarray(Wk[F].T).astype(BF16NP),
            wv=np.ascontiguousarray(Wv[F].T).astype(BF16NP),
            wo=np.ascontiguousarray(Wo[:, F].T).astype(BF16NP),
            bq=np.ascontiguousarray(
                (bq[F] * scale).reshape(4, 128).T, dtype=f32
            ),
        )
    in_maps = []
    for c in range(N_CORES):
        b, g = c // 2, c % 2
        qb, kb, vb = xT[b]
        in_maps.append(dict(qT=qb, kT=kb, vT=vb, **per_g[g]))
    return in_maps


def kernel(q, k, v, Wq, bq, Wk, bk, Wv, bv, Wo, bo):
    q, k, v = (np.asarray(x, np.float32) for x in (q, k, v))
    Wq, bq, Wk, bk = (np.asarray(x, np.float32) for x in (Wq, bq, Wk, bk))
    Wv, bv, Wo, bo = (np.asarray(x, np.float32) for x in (Wv, bv, Wo, bo))

    nc = get_nc(S)
    in_maps = _prep_in_maps(q, k, v, Wq, bq, Wk, Wv, Wo)
    res = run_bass_kernel_spmd(nc, in_maps, core_ids=list(range(N_CORES)))

    # bk drops out of softmax; bv folds into an effective output bias.
    bo_eff = (
        bo.astype(np.float64) + Wo.astype(np.float64) @ bv.astype(np.float64)
    ).astype(np.float32)
    out = np.empty((B, S, D), np.float32)
    for b in range(B):
        acc = res.results[2 * b]["outT"] + res.results[2 * b + 1]["outT"]
        out[b] = acc.T + bo_eff
    return out
